# revision 1
# baseline (speedup 1.0000x reference)
"""Trainium2 Bass kernel for a 12-layer GPT LM (CodeGPTLMHeadModel).

Sharding (8 NeuronCores, one chip):
  - Layer stack: tokens sharded. B=2 batches x 1024 tokens; cores 0-3 own
    batch 0, cores 4-7 batch 1; core c owns 256 contiguous tokens
    (chunks 2a, 2a+1 with a = c%4).  All weights replicated, streamed
    from HBM.  Attention: each core computes q/k/v for its local tokens,
    AllGathers K^T/V inside its 4-core batch group, then computes all 16
    heads for its local 256 queries (causal handled by a per-core mask
    input => uniform SPMD graph).
  - LM head: vocab sharded.  AllGather of the final hidden states across
    all 8 cores; each core computes a 6656-wide padded vocab slice.
  - Norm weights (ln1/ln2/lnf) are folded into the following matmul
    weights host-side; qn/kn/gate are applied on-device from replicated
    constant inputs.  Matmuls run in bf16 (f32 residual/psum).
"""

import numpy as np
import ml_dtypes

BF16 = ml_dtypes.bfloat16

L_ALL, B, T, D, H, HD, F, V = 12, 2, 1024, 1024, 16, 64, 4096, 50257
NCORE = 8
TLOC = 256            # tokens per core
QT = TLOC // 128      # 2 token tiles of 128
NKT = D // 128        # 8 contraction tiles over D
NFT = F // 128        # 32 tiles over F
VS = 6656             # padded vocab shard per core (13 * 512)
NVC = VS // 512       # 13 vocab chunks of 512
EPS = 1e-5
LO_ROWS = 25088       # embed split for int16 gather indices (max idx 25088)
HI_ROWS = V - LO_ROWS  # 25169

# kv bounce layout (bf16): one row per partition: k^T 2048 cols + v 2*1040 cols
KV_COLS = 2048 + 2 * 1040  # 4128


def build_nc(n_layers=L_ALL):
    from contextlib import ExitStack
    from concourse import bass, bacc, mybir, tile

    f32 = mybir.dt.float32
    bf = mybir.dt.bfloat16
    i16 = mybir.dt.int16
    AF = mybir.ActivationFunctionType

    nc = bacc.Bacc(None, target_bir_lowering=False, debug=False)

    # ---------------- external parameters (per-core shards) ----------------
    x0_d = nc.dram_tensor("x0", [TLOC, D], f32, kind="ExternalInput")
    maskt = nc.dram_tensor("maskt", [8, 128, TLOC], bf, kind="ExternalInput")
    wq_d = nc.dram_tensor("wq", [n_layers, D, D], bf, kind="ExternalInput")
    wk_d = nc.dram_tensor("wk", [n_layers, D, D], bf, kind="ExternalInput")
    wv_d = nc.dram_tensor("wv", [n_layers, D, D], bf, kind="ExternalInput")
    wo_d = nc.dram_tensor("wo", [n_layers, D, D], bf, kind="ExternalInput")
    w1_d = nc.dram_tensor("w1", [n_layers, D, F], bf, kind="ExternalInput")
    w2_d = nc.dram_tensor("w2", [n_layers, F, D], bf, kind="ExternalInput")
    qnk_d = nc.dram_tensor("qnk", [n_layers, 2, 128, D], bf, kind="ExternalInput")
    g_d = nc.dram_tensor("g", [n_layers, 128, H], f32, kind="ExternalInput")
    wlm_d = nc.dram_tensor("wlm", [D, VS], bf, kind="ExternalInput")
    out_d = nc.dram_tensor("out", [B * T, VS], bf, kind="ExternalOutput")

    id_np = np.eye(128, dtype=BF16)
    id_dram = nc.inline_tensor(id_np, name="id128")

    with tile.TileContext(nc) as tc, ExitStack() as ctx:
        ep = ctx.enter_context

        consts = ep(tc.tile_pool(name="consts", bufs=1))
        p_res = ep(tc.tile_pool(name="p_res", bufs=1))
        p_h = ep(tc.tile_pool(name="p_h", bufs=3))
        p_tr = ep(tc.tile_pool(name="p_tr", bufs=1))
        p_small = ep(tc.tile_pool(name="p_small", bufs=8))
        p_v = ep(tc.tile_pool(name="p_v", bufs=1))
        p_o2 = ep(tc.tile_pool(name="p_o2", bufs=1))
        p_uT = ep(tc.tile_pool(name="p_uT", bufs=1))
        p_slab = ep(tc.tile_pool(name="p_slab", bufs=1))
        p_w = ep(tc.tile_pool(name="p_w", bufs=3))
        p_lmh = ep(tc.tile_pool(name="p_lmh", bufs=1))
        p_out = ep(tc.tile_pool(name="p_out", bufs=2))
        p_qn = ep(tc.tile_pool(name="p_qn", bufs=2))
        p_p = ep(tc.tile_pool(name="p_p", bufs=3))
        ps = ep(tc.tile_pool(name="ps", bufs=8, space="PSUM"))
        dram = ep(tc.tile_pool(name="dram", bufs=2, space="DRAM"))

        # ---------------- constants into SBUF ----------------
        id_sb = consts.tile([128, 128], bf, tag="id", name="id")
        nc.sync.dma_start(id_sb[:], id_dram[:, :])
        mask_sb = consts.tile([128, 8, TLOC], bf, tag="mask", name="mask")
        nc.sync.dma_start(mask_sb[:], maskt[:, :, :].rearrange("t p q -> p t q"))
        eps_sb = consts.tile([128, 1], f32, tag="eps", name="eps")
        nc.vector.memset(eps_sb[:], EPS)

        # ---------------- embedding ----------------
        x = [p_res.tile([128, D], f32, tag=f"x{q}", name=f"x{q}") for q in range(QT)]

        for q in range(QT):
            nc.sync.dma_start(x[q][:], x0_d[q * 128:(q + 1) * 128, :])

        # ---------------- helpers ----------------
        def rms_to_hT(tag):
            """RMS-normalize x (token-major) -> h bf16 -> transposed hT[128,8,256]."""
            hT = p_tr.tile([128, NKT, TLOC], bf, tag=tag)
            for q in range(QT):
                sq = p_h.tile([128, D], bf, tag="sc", name="sq")
                ssq = p_small.tile([128, 1], f32, tag="ssq", name="ssq")
                nc.scalar.activation(sq[:], x[q][:], AF.Square, accum_out=ssq[:])
                std = p_small.tile([128, 1], f32, tag="std", name="std")
                nc.scalar.activation(std[:], ssq[:], AF.Sqrt, scale=1.0 / D, bias=eps_sb[:])
                inv = p_small.tile([128, 1], f32, tag="inv", name="inv")
                nc.vector.reciprocal(inv[:], std[:])
                h = p_h.tile([128, D], bf, tag="h", name="h")
                nc.vector.tensor_scalar_mul(h[:], x[q][:], inv[:])
                for d in range(NKT):
                    pt = ps.tile([128, 512], bf, tag="ps", name="ps")
                    nc.tensor.transpose(pt[:, :128], h[:, d * 128:(d + 1) * 128], id_sb[:])
                    nc.vector.tensor_copy(hT[:, d, q * 128:(q + 1) * 128], pt[:, :128])
            return hT

        def proj_qkv(hT, w_dram, l):
            """form-1 projection: out[128tok, D] psum pair per (q, ch)."""
            outs = {}
            for ch in range(2):
                pts = [ps.tile([128, 512], f32, tag="ps", name="ps") for _ in range(QT)]
                wt = p_w.tile([128, NKT, 512], bf, tag="w", name="w")
                nc.sync.dma_start(
                    wt[:], w_dram[l, :, ch * 512:(ch + 1) * 512]
                    .rearrange("(a p) c -> p a c", p=128)
                )
                for k in range(NKT):
                    for q in range(QT):
                        nc.tensor.matmul(
                            pts[q][:], hT[:, k, q * 128:(q + 1) * 128], wt[:, k, :],
                            start=(k == 0), stop=(k == NKT - 1),
                        )
                outs[ch] = pts
            return outs

        def qknorm_transpose(pq, qn_sb, which, tag):
            """QK-norm (token-major, from psum) + qn/kn apply + transpose.

            pq: dict ch -> [QT] psum tiles [128, 512] (= [128, 8, 64])
            returns qT [128, 8, 256] bf16 (partition = dim%128 within pairs of heads)
            """
            qT = p_tr.tile([128, NKT, TLOC], bf, tag=tag)
            for q in range(QT):
                ss = p_small.tile([128, H], f32, tag="ssqk", name="ssqk")
                for ch in range(2):
                    sqs = p_h.tile([128, 512], bf, tag="sc", name="sqs")
                    nc.scalar.activation(sqs[:], pq[ch][q][:], AF.Square)
                    nc.vector.tensor_reduce(
                        ss[:, ch * 8:(ch + 1) * 8],
                        sqs[:].rearrange("p (h d) -> p h d", d=HD),
                        axis=mybir.AxisListType.X, op=mybir.AluOpType.add,
                    )
                st = p_small.tile([128, H], f32, tag="stqk", name="stqk")
                nc.scalar.activation(st[:], ss[:], AF.Sqrt, scale=1.0 / HD, bias=eps_sb[:])
                iv = p_small.tile([128, H], f32, tag="ivqk", name="ivqk")
                nc.vector.reciprocal(iv[:], st[:])
                qh = p_h.tile([128, D], bf, tag="qh", name="qh")
                for ch in range(2):
                    tmp = p_h.tile([128, 512], f32, tag="sc", name="qtmp")
                    nc.vector.tensor_tensor(
                        tmp[:].rearrange("p (h d) -> p h d", d=HD),
                        pq[ch][q][:].rearrange("p (h d) -> p h d", d=HD),
                        iv[:, ch * 8:(ch + 1) * 8, None].to_broadcast((128, 8, HD)),
                        op=mybir.AluOpType.mult,
                    )
                    nc.vector.tensor_mul(
                        qh[:, ch * 512:(ch + 1) * 512], tmp[:],
                        qn_sb[:, which, ch * 512:(ch + 1) * 512],
                    )
                for d in range(NKT):
                    pt = ps.tile([128, 512], bf, tag="ps", name="ps")
                    nc.tensor.transpose(pt[:, :128], qh[:, d * 128:(d + 1) * 128], id_sb[:])
                    nc.vector.tensor_copy(qT[:, d, q * 128:(q + 1) * 128], pt[:, :128])
            return qT

        # ---------------- layers ----------------
        for l in range(n_layers):
            qn_sb = p_qn.tile([128, 2, D], bf, tag="qn", name="qn")
            nc.sync.dma_start(qn_sb[:], qnk_d[l, :, :, :].rearrange("a p d -> p a d"))
            g_sb = p_qn.tile([128, H], f32, tag="g", name="g")
            nc.sync.dma_start(g_sb[:], g_d[l, :, :])

            hT = rms_to_hT("hT")

            # --- K/V projections first; Q computed during the AllGather ---
            pk = proj_qkv(hT, wk_d, l)
            pv = proj_qkv(hT, wv_d, l)

            # v eviction: [128, 16, 65] with ones in col 64
            v_sb = []
            for q in range(QT):
                vt = p_v.tile([128, H, HD + 1], bf, tag=f"v{q}", name=f"v{q}")
                for ch in range(2):
                    nc.scalar.activation(
                        vt[:, ch * 8:(ch + 1) * 8, :HD],
                        pv[ch][q][:].rearrange("p (h d) -> p h d", d=HD),
                        AF.Copy,
                    )
                nc.vector.memset(vt[:, :, HD:], 1.0)
                v_sb.append(vt)

            kT = qknorm_transpose(pk, qn_sb, 1, "kT")

            # --- bounce K^T and V to DRAM, AllGather within batch group ---
            kv_in = dram.tile([128, KV_COLS], bf, tag="kv_in", name="kv_in")
            kv_out = dram.tile([4 * 128, KV_COLS], bf, tag="kv_out", name="kv_out")
            nc.sync.dma_start(kv_in[:, 0:2048], kT[:])
            for q in range(QT):
                nc.sync.dma_start(
                    kv_in[:, 2048 + q * 1040: 2048 + (q + 1) * 1040], v_sb[q][:]
                )
            nc.gpsimd.collective_compute(
                "AllGather", mybir.AluOpType.bypass,
                ins=[kv_in[:].opt()], outs=[kv_out[:].opt()],
                replica_groups=[[0, 1, 2, 3], [4, 5, 6, 7]],
            )
            pq = proj_qkv(hT, wq_d, l)
            qT = qknorm_transpose(pq, qn_sb, 0, "qT")

            kTf = p_slab.tile([128, 4 * NKT, TLOC], bf, tag="kTf", name="kTf")
            vf = p_slab.tile([128, 8, H, HD + 1], bf, tag="vf", name="vf")
            for s in range(4):
                nc.sync.dma_start(
                    kTf[:, s * 8:(s + 1) * 8, :],
                    kv_out[s * 128:(s + 1) * 128, 0:2048],
                )
                nc.sync.dma_start(
                    vf[:, 2 * s:2 * s + 2, :, :],
                    kv_out[s * 128:(s + 1) * 128, 2048:KV_COLS],
                )

            # --- attention: all 16 heads, 8 key tiles, local 256 queries ---
            o2 = [p_o2.tile([128, H, HD], f32, tag=f"o2{q}", name=f"o2{q}") for q in range(QT)]
            for h in range(H):
                po = ps.tile([HD + 1, 512], f32, tag="ps", name="ps")
                qr = qT[64 * (h % 2): 64 * (h % 2) + 64, h // 2, :]
                for t in range(8):
                    pss = ps.tile([128, 512], f32, tag="ps", name="ps")
                    lk = kTf[64 * (h % 2): 64 * (h % 2) + 64,
                             (t // 2) * 8 + h // 2,
                             (t % 2) * 128: (t % 2) * 128 + 128]
                    nc.tensor.matmul(pss[:, :TLOC], lk, qr, start=True, stop=True)
                    pe = p_p.tile([128, TLOC], bf, tag="pe", name="pe")
                    nc.scalar.activation(pe[:], pss[:, :TLOC], AF.Exp)
                    nc.vector.tensor_mul(pe[:], pe[:], mask_sb[:, t, :])
                    nc.tensor.matmul(
                        po[:, :TLOC], vf[:, t, h, :], pe[:],
                        start=(t == 0), stop=(t == 7),
                    )
                # epilogue: transpose, divide by sums, gate
                ot = p_p.tile([HD + 1, TLOC], bf, tag="ot", name="ot")
                nc.scalar.activation(ot[:], po[:, :TLOC], AF.Copy)
                for q in range(QT):
                    px = ps.tile([128, 512], bf, tag="ps", name="ps")
                    nc.tensor.transpose(
                        px[:, :HD + 1], ot[:, q * 128:(q + 1) * 128],
                        id_sb[:HD + 1, :HD + 1],
                    )
                    inv = p_small.tile([128, 1], f32, tag="ainv", name="ainv")
                    nc.vector.reciprocal(inv[:], px[:, HD:HD + 1])
                    ivg = p_small.tile([128, 1], f32, tag="aivg", name="aivg")
                    nc.vector.tensor_mul(ivg[:], inv[:], g_sb[:, h:h + 1])
                    nc.scalar.activation(
                        o2[q][:, h, :], px[:, :HD], AF.Copy, scale=ivg[:]
                    )

            # value residual + transpose for Wo
            o2T = p_tr.tile([128, NKT, TLOC], bf, tag="o2T", name="o2T")
            for q in range(QT):
                nc.vector.tensor_add(o2[q][:], o2[q][:], v_sb[q][:, :, :HD])
                o2b = p_h.tile([128, D], bf, tag="sc", name="o2b")
                nc.vector.tensor_copy(o2b[:], o2[q][:].rearrange("p h d -> p (h d)"))
                for d in range(NKT):
                    pt = ps.tile([128, 512], bf, tag="ps", name="ps")
                    nc.tensor.transpose(pt[:, :128], o2b[:, d * 128:(d + 1) * 128], id_sb[:])
                    nc.vector.tensor_copy(o2T[:, d, q * 128:(q + 1) * 128], pt[:, :128])

            # --- Wo: x += o2 @ Wo ---
            for ch in range(2):
                pts = [ps.tile([128, 512], f32, tag="ps", name="ps") for _ in range(QT)]
                wt = p_w.tile([128, NKT, 512], bf, tag="w", name="w")
                nc.sync.dma_start(
                    wt[:], wo_d[l, :, ch * 512:(ch + 1) * 512]
                    .rearrange("(a p) c -> p a c", p=128)
                )
                for k in range(NKT):
                    for q in range(QT):
                        nc.tensor.matmul(
                            pts[q][:], o2T[:, k, q * 128:(q + 1) * 128], wt[:, k, :],
                            start=(k == 0), stop=(k == NKT - 1),
                        )
                for q in range(QT):
                    nc.vector.tensor_add(
                        x[q][:, ch * 512:(ch + 1) * 512],
                        x[q][:, ch * 512:(ch + 1) * 512], pts[q][:],
                    )

            # --- MLP ---
            hmT = rms_to_hT("hmT")
            uT = p_uT.tile([128, NFT, TLOC], bf, tag="uT", name="uT")
            for fg in range(NFT // 4):
                w1t = p_w.tile([128, NKT, 4, 128], bf, tag="w", name="w1")
                nc.sync.dma_start(
                    w1t[:],
                    w1_d[l, :, fg * 512:(fg + 1) * 512]
                    .rearrange("(a p) (b f) -> p a b f", p=128, f=128),
                )
                for fi in range(4):
                    fc = fg * 4 + fi
                    pu = ps.tile([128, 512], f32, tag="ps", name="ps")
                    for k in range(NKT):
                        nc.tensor.matmul(
                            pu[:, :TLOC], w1t[:, k, fi, :], hmT[:, k, :],
                            start=(k == 0), stop=(k == NKT - 1),
                        )
                    nc.scalar.activation(uT[:, fc, :], pu[:, :TLOC], AF.Gelu)
            for ch in range(2):
                pts = [ps.tile([128, 512], f32, tag="ps", name="ps") for _ in range(QT)]
                for fgg in range(4):
                    w2t = p_w.tile([128, NKT, 512], bf, tag="w", name="w")
                    nc.sync.dma_start(
                        w2t[:], w2_d[l, fgg * 1024:(fgg + 1) * 1024,
                                     ch * 512:(ch + 1) * 512]
                        .rearrange("(a p) c -> p a c", p=128)
                    )
                    for ki in range(NKT):
                        fc = fgg * 8 + ki
                        for q in range(QT):
                            nc.tensor.matmul(
                                pts[q][:], uT[:, fc, q * 128:(q + 1) * 128],
                                w2t[:, ki, :],
                                start=(fc == 0), stop=(fc == NFT - 1),
                            )
                for q in range(QT):
                    nc.vector.tensor_add(
                        x[q][:, ch * 512:(ch + 1) * 512],
                        x[q][:, ch * 512:(ch + 1) * 512], pts[q][:],
                    )

        # ---------------- final norm + LM head ----------------
        hfT = rms_to_hT("hT")
        lm_in = dram.tile([128, 2048], bf, tag="lm_in", name="lm_in")
        lm_out = dram.tile([8 * 128, 2048], bf, tag="lm_out", name="lm_out", addr_space="Shared")
        nc.sync.dma_start(lm_in[:, :], hfT[:])
        nc.gpsimd.collective_compute(
            "AllGather", mybir.AluOpType.bypass,
            ins=[lm_in[:].opt()], outs=[lm_out[:].opt()],
            replica_groups=[[0, 1, 2, 3, 4, 5, 6, 7]],
        )
        hfa = p_lmh.tile([128, 8 * NKT, TLOC], bf, tag="hfT", name="hfT")
        for s in range(8):
            nc.sync.dma_start(
                hfa[:, s * 8:(s + 1) * 8, :],
                lm_out[s * 128:(s + 1) * 128, :],
            )
        for vt in range(NVC):
            wl = p_w.tile([128, NKT, 512], bf, tag="w", name="wlm")
            nc.sync.dma_start(
                wl[:],
                wlm_d[:, vt * 512:(vt + 1) * 512]
                .rearrange("(a p) c -> p a c", p=128),
            )
            for tg in range(4):
                ob = p_out.tile([128, 4, 512], bf, tag="ob", name="ob")
                for ti in range(4):
                    tt = tg * 4 + ti
                    pl = ps.tile([128, 512], f32, tag="ps", name="ps")
                    for k in range(NKT):
                        nc.tensor.matmul(
                            pl[:],
                            hfa[:, (tt // 2) * 8 + k,
                                (tt % 2) * 128:(tt % 2) * 128 + 128],
                            wl[:, k, :],
                            start=(k == 0), stop=(k == NKT - 1),
                        )
                    nc.scalar.activation(ob[:, ti, :], pl[:], AF.Copy)
                nc.sync.dma_start(
                    out_d[tg * 512:(tg + 1) * 512, vt * 512:(vt + 1) * 512]
                    .rearrange("(b p) c -> p b c", p=128),
                    ob[:],
                )
    nc.compile()
    return nc


# ---------------------------------------------------------------------------
# host side
# ---------------------------------------------------------------------------

def _prep_inputs(inputs, n_layers=L_ALL):
    ids = np.asarray(inputs["input_ids"])
    embed = np.asarray(inputs["embed"], np.float32)
    pos = np.asarray(inputs["pos_embed"], np.float32)
    ln1 = np.asarray(inputs["ln1_w"], np.float32)
    ln2 = np.asarray(inputs["ln2_w"], np.float32)
    qn = np.asarray(inputs["qn_w"], np.float32)
    kn = np.asarray(inputs["kn_w"], np.float32)
    gate = np.asarray(inputs["gate"], np.float32)
    lnf = np.asarray(inputs["lnf_w"], np.float32)

    wq = (ln1[:, :, None] * np.asarray(inputs["Wq"], np.float32)).astype(BF16)
    wk = (ln1[:, :, None] * np.asarray(inputs["Wk"], np.float32)).astype(BF16)
    wv = (ln1[:, :, None] * np.asarray(inputs["Wv"], np.float32)).astype(BF16)
    wo = np.asarray(inputs["Wo"], np.float32).astype(BF16)
    w1 = (ln2[:, :, None] * np.asarray(inputs["W1"], np.float32)).astype(BF16)
    w2 = np.asarray(inputs["W2"], np.float32).astype(BF16)
    wlm_full = lnf[:, None] * np.asarray(inputs["Wlm"], np.float32)
    wlm_pad = np.zeros((D, VS * NCORE), np.float32)
    wlm_pad[:, :V] = wlm_full
    wlm_pad = wlm_pad.astype(BF16)

    # qn/kn replicated [L, 2, 128, D]; 1/sqrt(HD) folded into kn side
    qnk = np.zeros((n_layers, 2, 128, D), np.float32)
    for l in range(n_layers):
        qnk[l, 0, :, :] = np.tile(qn[l], H)[None, :]
        qnk[l, 1, :, :] = np.tile(kn[l] / np.sqrt(HD), H)[None, :]
    qnk = qnk.astype(BF16)
    g_rep = np.broadcast_to(gate[:n_layers, None, :], (n_layers, 128, H)).copy()

    # zero row at index 0; real rows shifted by +1
    in_maps = []
    for c in range(NCORE):
        b, a = divmod(c, 4)
        t0 = a * TLOC
        toks = ids[b, t0:t0 + TLOC].astype(np.int64)
        x0 = embed[toks] + pos[t0:t0 + TLOC]
        # causal mask [8 key tiles, 128 key pos, 256 query pos]
        kg = np.arange(T).reshape(8, 128)
        qg = t0 + np.arange(TLOC)
        mask = (kg[:, :, None] <= qg[None, None, :]).astype(BF16)
        in_maps.append({
            "x0": x0.astype(np.float32),
            "maskt": mask,
            "wq": wq[:n_layers], "wk": wk[:n_layers], "wv": wv[:n_layers],
            "wo": wo[:n_layers], "w1": w1[:n_layers], "w2": w2[:n_layers],
            "qnk": qnk, "g": g_rep.astype(np.float32),
            "wlm": wlm_pad[:, c * VS:(c + 1) * VS],
        })
    return in_maps


_NC_CACHE = {}


def _get_nc(n_layers=L_ALL):
    if n_layers not in _NC_CACHE:
        _NC_CACHE[n_layers] = build_nc(n_layers)
    return _NC_CACHE[n_layers]


def _install_profile_hook():
    """Recreate antenv.axon_hooks with an NTFF profile hook via ctypes."""
    import sys as _sys, types, ctypes, contextlib, os
    try:
        import antenv.axon_hooks  # noqa: F401
        return
    except ImportError:
        pass
    so_path = os.environ.get("PJRT_LIBRARY_PATH", "/opt/axon/libaxon_pjrt.so")
    lib = ctypes.CDLL(so_path)
    if not hasattr(lib, "axon_start_nrt_profile"):
        return
    lib.axon_start_nrt_profile.argtypes = [ctypes.POINTER(ctypes.c_int64), ctypes.c_size_t]
    lib.axon_start_nrt_profile.restype = ctypes.c_int64
    lib.axon_stop_nrt_profile.argtypes = [ctypes.c_char_p]
    lib.axon_stop_nrt_profile.restype = ctypes.c_int64

    @contextlib.contextmanager
    def _hook(output_dir, device_ids):
        import jax
        jax.devices()
        if device_ids:
            ids = (ctypes.c_int64 * len(device_ids))(*device_ids)
            rc = lib.axon_start_nrt_profile(ids, len(device_ids))
        else:
            rc = lib.axon_start_nrt_profile(None, 0)
        if rc != 0:
            raise RuntimeError(f"axon_start_nrt_profile rc={rc}")
        try:
            yield
        finally:
            n = lib.axon_stop_nrt_profile(str(output_dir).encode())
            print(f"profile: {n} file(s) written to {output_dir}")

    import antenv
    mod = types.ModuleType("antenv.axon_hooks")
    _state = {"hook": _hook}
    mod.set_axon_ntff_profile_hook = lambda h: _state.__setitem__("hook", h)
    mod.get_axon_ntff_profile_hook = lambda: _state["hook"]
    _sys.modules["antenv.axon_hooks"] = mod
    antenv.axon_hooks = mod


def run(inputs, n_layers=L_ALL, trace=False):
    from concourse.bass_utils import run_bass_kernel_spmd
    if trace:
        _install_profile_hook()
    nc = _get_nc(n_layers)
    in_maps = _prep_inputs(inputs, n_layers)
    res = run_bass_kernel_spmd(
        nc, in_maps, core_ids=list(range(NCORE)), trace=trace,
    )
    outs = [np.asarray(r["out"], dtype=np.float32) for r in res.results]
    logits = np.concatenate(outs, axis=1)[:, :V]
    return logits.reshape(B, T, V), res


def kernel(**inputs):
    logits, _ = run(inputs)
    return logits




# revision 9
# speedup vs baseline: 1.1893x; 1.1893x over previous
"""Trainium2 Bass kernel for a 12-layer GPT LM (CodeGPTLMHeadModel).

Sharding (8 NeuronCores, one chip):
  - Tokens resharded so every core owns the SAME 128-token range of BOTH
    batches: core c owns tokens [c*128,(c+1)*128) of batch 0 and batch 1
    (256 local tokens).  This makes the per-layer K/V exchange a single
    8-core AllGather with a Shared output buffer (the fast collective
    path) with fully uniform SPMD readback.
  - K^T and Q are exchanged/kept in fp8e4 (post QK-norm, RMS~1, and the
    softmax path is scale-invariant / smoothing, so fp8 noise is safe).
    V stays bf16.
  - LM head: vocab sharded; AllGather of final hiddens across all 8
    cores (Shared), each core computes a 6656-wide padded vocab slice.
  - Norm weights (ln1/ln2/lnf) folded into the following matmul weights
    host-side; qn/kn/gate applied on-device.  Matmuls in bf16 (f32
    psum); attention QK in fp8.
"""

import numpy as np
import ml_dtypes

BF16 = ml_dtypes.bfloat16

L_ALL, B, T, D, H, HD, F, V = 12, 2, 1024, 1024, 16, 64, 4096, 50257
NCORE = 8
TLOC = 256            # tokens per core: 128 of batch0 + 128 of batch1
QT = 2                # 2 token tiles of 128 (one per batch)
NKT = D // 128        # 8 contraction tiles over D
NFT = F // 128        # 32 tiles over F
VS = 6656             # padded vocab shard per core (13 * 512)
NVC = VS // 512       # 13 vocab chunks of 512
EPS = 1e-5

# kv bounce layout (fp8 bytes): kT 2048 + v bf16 2*1040*2 bytes
KV_K = 2048                      # kT fp8 cols
KV_V = 2 * 1040 * 2              # v bf16 bytes (2 tiles x 16h x 65 x 2B)
KV_COLS = KV_K + KV_V            # 6208 fp8 cols per row


def build_nc(n_layers=L_ALL):
    from contextlib import ExitStack
    from concourse import bass, bacc, mybir, tile

    f32 = mybir.dt.float32
    bf = mybir.dt.bfloat16
    f8 = mybir.dt.float8e4
    AF = mybir.ActivationFunctionType

    nc = bacc.Bacc(None, target_bir_lowering=False, debug=False)

    # ---------------- external parameters (per-core shards) ----------------
    x0_d = nc.dram_tensor("x0", [TLOC, D], f32, kind="ExternalInput")
    maskt = nc.dram_tensor("maskt", [8, 128, 128], bf, kind="ExternalInput")
    wq_d = nc.dram_tensor("wq", [n_layers, D, D], bf, kind="ExternalInput")
    wk_d = nc.dram_tensor("wk", [n_layers, D, D], bf, kind="ExternalInput")
    wv_d = nc.dram_tensor("wv", [n_layers, D, D], bf, kind="ExternalInput")
    wo_d = nc.dram_tensor("wo", [n_layers, D, D], bf, kind="ExternalInput")
    w1_d = nc.dram_tensor("w1", [n_layers, D, F], bf, kind="ExternalInput")
    w2_d = nc.dram_tensor("w2", [n_layers, F, D], bf, kind="ExternalInput")
    qnk_d = nc.dram_tensor("qnk", [n_layers, 2, 128, D], bf, kind="ExternalInput")
    g2_d = nc.dram_tensor("g2", [n_layers, 128, 2 * H], f32, kind="ExternalInput")
    wlm_d = nc.dram_tensor("wlm", [D, VS], bf, kind="ExternalInput")
    out_d = nc.dram_tensor("out", [B * T, VS], bf, kind="ExternalOutput")

    id_np = np.eye(128, dtype=BF16)
    id_dram = nc.inline_tensor(id_np, name="id128")

    with tile.TileContext(nc) as tc, ExitStack() as ctx:
        ep = ctx.enter_context

        consts = ep(tc.tile_pool(name="consts", bufs=1))
        p_res = ep(tc.tile_pool(name="p_res", bufs=1))
        p_h = ep(tc.tile_pool(name="p_h", bufs=2))
        p_tr = ep(tc.tile_pool(name="p_tr", bufs=2))
        p_qk = ep(tc.tile_pool(name="p_qk", bufs=2))
        p_small = ep(tc.tile_pool(name="p_small", bufs=8))
        p_v = ep(tc.tile_pool(name="p_v", bufs=3))
        p_o2 = ep(tc.tile_pool(name="p_o2", bufs=2))
        p_pe = ep(tc.tile_pool(name="p_pe", bufs=3))
        p_ot = ep(tc.tile_pool(name="p_ot", bufs=2))
        p_slab = ep(tc.tile_pool(name="p_slab", bufs=1))
        p_w = ep(tc.tile_pool(name="p_w", bufs=2))
        p_uT = ep(tc.tile_pool(name="p_uT", bufs=1))
        p_qn = ep(tc.tile_pool(name="p_qn", bufs=2))
        p_out = ep(tc.tile_pool(name="p_out", bufs=2))
        ps = ep(tc.tile_pool(name="ps", bufs=2, space="PSUM"))
        dram = ep(tc.tile_pool(name="dram", bufs=2, space="DRAM"))

        # ---------------- constants into SBUF ----------------
        id_sb = consts.tile([128, 128], bf, tag="id", name="id")
        nc.sync.dma_start(id_sb[:], id_dram[:, :])
        mask_sb = consts.tile([128, 8, 128], bf, tag="mask", name="mask")
        nc.sync.dma_start(mask_sb[:], maskt[:, :, :].rearrange("s p q -> p s q"))
        eps_sb = consts.tile([128, 1], f32, tag="eps", name="eps")
        nc.vector.memset(eps_sb[:], EPS)

        # ---------------- embedding ----------------
        x = [p_res.tile([128, D], f32, tag=f"x{q}", name=f"x{q}") for q in range(QT)]
        for q in range(QT):
            nc.sync.dma_start(x[q][:], x0_d[q * 128:(q + 1) * 128, :])

        # ---------------- helpers ----------------
        def rms_to_hT(tag):
            """RMS-normalize x (token-major) -> h bf16 -> transposed hT[128,8,256]."""
            hT = p_tr.tile([128, NKT, TLOC], bf, tag="hT", name=tag)
            for q in range(QT):
                sq = p_h.tile([128, D], bf, tag="sq", name="sq")
                ssq = p_small.tile([128, 1], f32, tag="ssq", name="ssq")
                nc.scalar.activation(sq[:], x[q][:], AF.Square, accum_out=ssq[:])
                std = p_small.tile([128, 1], f32, tag="std", name="std")
                nc.scalar.activation(std[:], ssq[:], AF.Sqrt, scale=1.0 / D, bias=eps_sb[:])
                inv = p_small.tile([128, 1], f32, tag="inv", name="inv")
                nc.vector.reciprocal(inv[:], std[:])
                h = p_h.tile([128, D], bf, tag="h", name="h")
                nc.vector.tensor_scalar_mul(h[:], x[q][:], inv[:])
                pt = ps.tile([128, NKT, 128], bf, tag="tr", name="pt")
                for d in range(NKT):
                    nc.tensor.transpose(pt[:, d, :], h[:, d * 128:(d + 1) * 128], id_sb[:])
                nc.vector.tensor_copy(hT[:, :, q * 128:(q + 1) * 128], pt[:])
            return hT

        def proj(hT, w_dram, l):
            """x-proj: per q-tile one psum tile [128tok, 1024 outdim]."""
            wt = p_w.tile([128, NKT, D], bf, tag="w", name="w")
            nc.sync.dma_start(
                wt[:], w_dram[l, :, :].rearrange("(a p) c -> p a c", p=128)
            )
            outs = []
            for q in range(QT):
                pt = ps.tile([128, D], f32, tag="big", name="pbig")
                for ch in range(2):
                    for k in range(NKT):
                        nc.tensor.matmul(
                            pt[:, ch * 512:(ch + 1) * 512],
                            hT[:, k, q * 128:(q + 1) * 128],
                            wt[:, k, ch * 512:(ch + 1) * 512],
                            start=(k == 0), stop=(k == NKT - 1),
                        )
                outs.append(pt)
            return outs

        def qknorm_transpose(pq, qn_sb, which, tag):
            """QK-norm from psum [128,1024] + qn/kn apply -> fp8 transposed [128,8,256]."""
            qT = p_qk.tile([128, NKT, TLOC], f8, tag=tag, name=tag)
            for q in range(QT):
                sqs = p_h.tile([128, D], bf, tag="sq", name="sqs")
                nc.scalar.activation(sqs[:], pq[q][:], AF.Square)
                ss = p_small.tile([128, H], f32, tag="ss", name="ss")
                nc.vector.tensor_reduce(
                    ss[:], sqs[:].rearrange("p (h d) -> p h d", d=HD),
                    axis=mybir.AxisListType.X, op=mybir.AluOpType.add,
                )
                st = p_small.tile([128, H], f32, tag="st", name="st")
                nc.scalar.activation(st[:], ss[:], AF.Sqrt, scale=1.0 / HD, bias=eps_sb[:])
                iv = p_small.tile([128, H], f32, tag="iv", name="iv")
                nc.vector.reciprocal(iv[:], st[:])
                qh_t = p_h.tile([128, D], bf, tag="h", name="qh_t")
                nc.vector.tensor_tensor(
                    qh_t[:].rearrange("p (h d) -> p h d", d=HD),
                    pq[q][:].rearrange("p (h d) -> p h d", d=HD),
                    iv[:, :, None].to_broadcast((128, H, HD)),
                    op=mybir.AluOpType.mult,
                )
                qh = p_h.tile([128, D], bf, tag="qh", name="qh")
                nc.vector.tensor_mul(qh[:], qh_t[:], qn_sb[:, which, :])
                pt = ps.tile([128, NKT, 128], bf, tag="tr", name="ptq")
                for d in range(NKT):
                    nc.tensor.transpose(pt[:, d, :], qh[:, d * 128:(d + 1) * 128], id_sb[:])
                nc.vector.tensor_copy(qT[:, :, q * 128:(q + 1) * 128], pt[:])
            return qT

        # ---------------- layers ----------------
        for l in range(n_layers):
            qn_sb = p_qn.tile([128, 2, D], bf, tag="qn", name="qn")
            nc.sync.dma_start(qn_sb[:], qnk_d[l, :, :, :].rearrange("a p d -> p a d"))
            g2_sb = p_qn.tile([128, 2 * H], f32, tag="g2", name="g2")
            nc.sync.dma_start(g2_sb[:], g2_d[l, :, :])

            hT = rms_to_hT("hT")

            # --- K/V projections first; Q during the AllGather ---
            pk = proj(hT, wk_d, l)
            kT = qknorm_transpose(pk, qn_sb, 1, "kT")
            pv = proj(hT, wv_d, l)
            v_sb = []
            for q in range(QT):
                vt = p_v.tile([128, H, HD + 1], bf, tag="vt", name=f"v{q}")
                nc.scalar.activation(
                    vt[:, :, :HD],
                    pv[q][:].rearrange("p (h d) -> p h d", d=HD),
                    AF.Copy,
                )
                nc.vector.memset(vt[:, :, HD:], 1.0)
                v_sb.append(vt)

            # --- bounce K^T (fp8) + V (bf16 bytes) to DRAM; 8-core Shared AllGather ---
            kv_in = dram.tile([128, KV_COLS], f8, tag="kv_in", name="kv_in")
            kv_out = dram.tile([8 * 128, KV_COLS], f8, tag="kv_out", name="kv_out",
                               addr_space="Shared")
            nc.sync.dma_start(
                kv_in[:, 0:KV_K].rearrange("p (a c) -> p a c", a=NKT), kT[:]
            )
            for q in range(QT):
                nc.sync.dma_start(
                    kv_in[:, KV_K + q * 2080: KV_K + (q + 1) * 2080]
                    .rearrange("p (h c) -> p h c", h=H),
                    v_sb[q][:].bitcast(f8),
                )
            nc.gpsimd.collective_compute(
                "AllGather", mybir.AluOpType.bypass,
                ins=[kv_in[:].opt()], outs=[kv_out[:].opt()],
                replica_groups=[[0, 1, 2, 3, 4, 5, 6, 7]],
            )

            # --- Q projection + norm during the AllGather ---
            pq = proj(hT, wq_d, l)
            qT = qknorm_transpose(pq, qn_sb, 0, "qT")

            # --- gather-back: kT slabs first (QK can start), then V ---
            kTf = p_slab.tile([128, 8, NKT, TLOC], f8, tag="kTf", name="kTf")
            vf = p_slab.tile([128, 8, 2, H, HD + 1], bf, tag="vf", name="vf")
            for s in range(8):
                nc.sync.dma_start(
                    kTf[:, s, :, :],
                    kv_out[s * 128:(s + 1) * 128, 0:KV_K]
                    .rearrange("p (a c) -> p a c", a=NKT),
                )
            for s in range(8):
                nc.sync.dma_start(
                    vf[:, s, :, :, :],
                    kv_out[s * 128:(s + 1) * 128, KV_K:KV_COLS]
                    .bitcast(bf).rearrange("p (b h c) -> p b h c", b=2, h=H),
                )

            # --- attention: 4 groups of 4 heads; per (h,b): QK batch -> exp -> mask -> AV ---
            o2 = [p_o2.tile([128, H, HD], bf, tag=f"o2{q}", name=f"o2{q}")
                  for q in range(QT)]
            for grp in range(4):
                po = ps.tile([HD + 1, 8, 128], f32, tag="po", name="po", bufs=1)
                for hi in range(4):
                    h = grp * 4 + hi
                    hp = 64 * (h % 2)
                    dt_ = h // 2
                    for b in range(QT):
                        pqk = ps.tile([128, 8, 128], f32, tag="big", name="pqk")
                        qr = qT[hp:hp + 64, dt_, b * 128:(b + 1) * 128]
                        for s in range(8):
                            nc.tensor.matmul(
                                pqk[:, s, :], kTf[hp:hp + 64, s, dt_, b * 128:(b + 1) * 128],
                                qr, start=True, stop=True,
                            )
                        pe = p_pe.tile([128, 8, 128], bf, tag="pe", name="pe")
                        nc.scalar.activation(pe[:], pqk[:], AF.Exp)
                        nc.vector.tensor_mul(pe[:], pe[:], mask_sb[:])
                        for s in range(8):
                            nc.tensor.matmul(
                                po[:, 2 * hi + b, :], vf[:, s, b, h, :], pe[:, s, :],
                                start=(s == 0), stop=(s == 7),
                            )
                # epilogue for the 4-head group
                ot = p_ot.tile([HD + 1, 8, 128], bf, tag="ot", name="ot")
                nc.vector.tensor_copy(ot[:], po[:])
                px = ps.tile([128, 8, HD + 2], bf, tag="tr", name="px")
                for sl in range(8):
                    nc.tensor.transpose(
                        px[:, sl, :HD + 1], ot[:, sl, :], id_sb[:HD + 1, :HD + 1]
                    )
                iv_a = p_small.tile([128, 8], f32, tag="iva", name="iva")
                nc.vector.reciprocal(iv_a[:], px[:, :, HD])
                ivg = p_small.tile([128, 8], f32, tag="ivg", name="ivg")
                nc.vector.tensor_mul(ivg[:], iv_a[:], g2_sb[:, 8 * grp:8 * grp + 8])
                for b in range(QT):
                    nc.vector.tensor_tensor(
                        o2[b][:, grp * 4:(grp + 1) * 4, :],
                        px[:, b::2, :HD],
                        ivg[:, b::2, None].to_broadcast((128, 4, HD)),
                        op=mybir.AluOpType.mult,
                    )

            # value residual + transpose for Wo
            o2T = p_tr.tile([128, NKT, TLOC], bf, tag="hT", name="o2T")
            for q in range(QT):
                nc.vector.tensor_add(o2[q][:], o2[q][:], v_sb[q][:, :, :HD])
                pt = ps.tile([128, NKT, 128], bf, tag="tr", name="pto")
                for d in range(NKT):
                    nc.tensor.transpose(
                        pt[:, d, :],
                        o2[q][:].rearrange("p h d -> p (h d)")[:, d * 128:(d + 1) * 128],
                        id_sb[:],
                    )
                nc.vector.tensor_copy(o2T[:, :, q * 128:(q + 1) * 128], pt[:])

            # --- Wo: x += o2 @ Wo ---
            po_ = proj(o2T, wo_d, l)
            for q in range(QT):
                nc.vector.tensor_add(x[q][:], x[q][:], po_[q][:])

            # --- MLP ---
            hmT = rms_to_hT("hmT")
            uT = p_uT.tile([128, NFT, TLOC], bf, tag="uT", name="uT")
            for fg in range(NFT // 4):
                w1t = p_w.tile([128, NKT, 4, 128], bf, tag="w", name="w1")
                nc.sync.dma_start(
                    w1t[:],
                    w1_d[l, :, fg * 512:(fg + 1) * 512]
                    .rearrange("(a p) (b f) -> p a b f", p=128, f=128),
                )
                pu = ps.tile([128, 4, TLOC], f32, tag="big", name="pu")
                for fi in range(4):
                    for k in range(NKT):
                        nc.tensor.matmul(
                            pu[:, fi, :], w1t[:, k, fi, :], hmT[:, k, :],
                            start=(k == 0), stop=(k == NKT - 1),
                        )
                nc.scalar.activation(uT[:, fg * 4:(fg + 1) * 4, :], pu[:], AF.Gelu)
            pw2 = [ps.tile([128, D], f32, tag="big", name=f"pw2_{q}")
                   for q in range(QT)]
            for fgg in range(4):
                w2t = p_w.tile([128, NKT, D], bf, tag="w", name="w2")
                nc.sync.dma_start(
                    w2t[:], w2_d[l, fgg * 1024:(fgg + 1) * 1024, :]
                    .rearrange("(a p) c -> p a c", p=128)
                )
                for ki in range(NKT):
                    fc = fgg * 8 + ki
                    for q in range(QT):
                        for ch in range(2):
                            nc.tensor.matmul(
                                pw2[q][:, ch * 512:(ch + 1) * 512],
                                uT[:, fc, q * 128:(q + 1) * 128],
                                w2t[:, ki, ch * 512:(ch + 1) * 512],
                                start=(fc == 0), stop=(fc == NFT - 1),
                            )
            for q in range(QT):
                nc.vector.tensor_add(x[q][:], x[q][:], pw2[q][:])

        # ---------------- final norm + LM head ----------------
        hfT = rms_to_hT("hfT")
        lm_in = dram.tile([128, 2048], bf, tag="lm_in", name="lm_in")
        lm_out = dram.tile([8 * 128, 2048], bf, tag="lm_out", name="lm_out",
                           addr_space="Shared")
        nc.sync.dma_start(lm_in[:, :], hfT[:])
        nc.gpsimd.collective_compute(
            "AllGather", mybir.AluOpType.bypass,
            ins=[lm_in[:].opt()], outs=[lm_out[:].opt()],
            replica_groups=[[0, 1, 2, 3, 4, 5, 6, 7]],
        )
        hfa = p_slab.tile([128, 8 * NKT, TLOC], bf, tag="vf", name="hfa")
        for s in range(8):
            nc.sync.dma_start(
                hfa[:, s * 8:(s + 1) * 8, :],
                lm_out[s * 128:(s + 1) * 128, :],
            )
        for vt_ in range(NVC):
            wl = p_w.tile([128, NKT, 512], bf, tag="w", name="wlm")
            nc.sync.dma_start(
                wl[:],
                wlm_d[:, vt_ * 512:(vt_ + 1) * 512]
                .rearrange("(a p) c -> p a c", p=128),
            )
            for tg in range(4):
                ob = p_out.tile([128, 4, 512], bf, tag="ob", name="ob")
                for tp in range(2):
                    pl = ps.tile([128, 1024], f32, tag="big", name="pl")
                    for ti in range(2):
                        tt = tg * 4 + tp * 2 + ti
                        for k in range(NKT):
                            nc.tensor.matmul(
                                pl[:, ti * 512:(ti + 1) * 512],
                                hfa[:, (tt // 2) * 8 + k,
                                    (tt % 2) * 128:(tt % 2) * 128 + 128],
                                wl[:, k, :],
                                start=(k == 0), stop=(k == NKT - 1),
                            )
                    if tp == 0:
                        nc.scalar.activation(
                            ob[:].rearrange("p a c -> p (a c)")[:, 0:1024],
                            pl[:], AF.Copy,
                        )
                    else:
                        nc.vector.tensor_copy(
                            ob[:].rearrange("p a c -> p (a c)")[:, 1024:2048],
                            pl[:],
                        )
                nc.sync.dma_start(
                    out_d[tg * 512:(tg + 1) * 512, vt_ * 512:(vt_ + 1) * 512]
                    .rearrange("(b p) c -> p b c", p=128),
                    ob[:],
                )
    nc.compile()
    return nc


# ---------------------------------------------------------------------------
# host side
# ---------------------------------------------------------------------------

def _prep_inputs(inputs, n_layers=L_ALL):
    ids = np.asarray(inputs["input_ids"])
    embed = np.asarray(inputs["embed"], np.float32)
    pos = np.asarray(inputs["pos_embed"], np.float32)
    ln1 = np.asarray(inputs["ln1_w"], np.float32)
    ln2 = np.asarray(inputs["ln2_w"], np.float32)
    qn = np.asarray(inputs["qn_w"], np.float32)
    kn = np.asarray(inputs["kn_w"], np.float32)
    gate = np.asarray(inputs["gate"], np.float32)
    lnf = np.asarray(inputs["lnf_w"], np.float32)

    wq = (ln1[:, :, None] * np.asarray(inputs["Wq"], np.float32)).astype(BF16)
    wk = (ln1[:, :, None] * np.asarray(inputs["Wk"], np.float32)).astype(BF16)
    wv = (ln1[:, :, None] * np.asarray(inputs["Wv"], np.float32)).astype(BF16)
    wo = np.asarray(inputs["Wo"], np.float32).astype(BF16)
    w1 = (ln2[:, :, None] * np.asarray(inputs["W1"], np.float32)).astype(BF16)
    w2 = np.asarray(inputs["W2"], np.float32).astype(BF16)
    wlm_full = lnf[:, None] * np.asarray(inputs["Wlm"], np.float32)
    wlm_pad = np.zeros((D, VS * NCORE), np.float32)
    wlm_pad[:, :V] = wlm_full
    wlm_pad = wlm_pad.astype(BF16)

    # qn/kn replicated [L, 2, 128, D]; 1/sqrt(HD) folded into kn side
    qnk = np.zeros((n_layers, 2, 128, D), np.float32)
    for l in range(n_layers):
        qnk[l, 0, :, :] = np.tile(qn[l], H)[None, :]
        qnk[l, 1, :, :] = np.tile(kn[l] / np.sqrt(HD), H)[None, :]
    qnk = qnk.astype(BF16)
    # per-head gate duplicated for (h, b) epilogue slots
    g2 = np.broadcast_to(
        np.repeat(gate[:n_layers], 2, axis=-1)[:, None, :], (n_layers, 128, 2 * H)
    ).copy().astype(np.float32)

    in_maps = []
    for c in range(NCORE):
        sl = slice(c * 128, (c + 1) * 128)
        t0 = c * 128
        x0 = np.concatenate(
            [embed[ids[b, sl].astype(np.int64)] + pos[sl] for b in range(B)], axis=0
        )
        # causal mask [8 key slices, 128 key pos, 128 query pos]
        kg = np.arange(T).reshape(8, 128)
        qg = t0 + np.arange(128)
        mask = (kg[:, :, None] <= qg[None, None, :]).astype(BF16)
        in_maps.append({
            "x0": x0.astype(np.float32),
            "maskt": mask,
            "wq": wq[:n_layers], "wk": wk[:n_layers], "wv": wv[:n_layers],
            "wo": wo[:n_layers], "w1": w1[:n_layers], "w2": w2[:n_layers],
            "qnk": qnk, "g2": g2,
            "wlm": wlm_pad[:, c * VS:(c + 1) * VS],
        })
    return in_maps


_NC_CACHE = {}


def _get_nc(n_layers=L_ALL):
    if n_layers not in _NC_CACHE:
        _NC_CACHE[n_layers] = build_nc(n_layers)
    return _NC_CACHE[n_layers]


def _install_profile_hook():
    """Recreate antenv.axon_hooks with an NTFF profile hook via ctypes."""
    import sys as _sys, types, ctypes, contextlib, os
    try:
        import antenv.axon_hooks  # noqa: F401
        return
    except ImportError:
        pass
    so_path = os.environ.get("PJRT_LIBRARY_PATH", "/opt/axon/libaxon_pjrt.so")
    lib = ctypes.CDLL(so_path)
    if not hasattr(lib, "axon_start_nrt_profile"):
        return
    lib.axon_start_nrt_profile.argtypes = [ctypes.POINTER(ctypes.c_int64), ctypes.c_size_t]
    lib.axon_start_nrt_profile.restype = ctypes.c_int64
    lib.axon_stop_nrt_profile.argtypes = [ctypes.c_char_p]
    lib.axon_stop_nrt_profile.restype = ctypes.c_int64

    @contextlib.contextmanager
    def _hook(output_dir, device_ids):
        import jax
        jax.devices()
        if device_ids:
            ids = (ctypes.c_int64 * len(device_ids))(*device_ids)
            rc = lib.axon_start_nrt_profile(ids, len(device_ids))
        else:
            rc = lib.axon_start_nrt_profile(None, 0)
        if rc != 0:
            raise RuntimeError(f"axon_start_nrt_profile rc={rc}")
        try:
            yield
        finally:
            n = lib.axon_stop_nrt_profile(str(output_dir).encode())
            print(f"profile: {n} file(s) written to {output_dir}")

    import antenv
    mod = types.ModuleType("antenv.axon_hooks")
    _state = {"hook": _hook}
    mod.set_axon_ntff_profile_hook = lambda h: _state.__setitem__("hook", h)
    mod.get_axon_ntff_profile_hook = lambda: _state["hook"]
    _sys.modules["antenv.axon_hooks"] = mod
    antenv.axon_hooks = mod


def run(inputs, n_layers=L_ALL, trace=False):
    from concourse.bass_utils import run_bass_kernel_spmd
    if trace:
        _install_profile_hook()
    nc = _get_nc(n_layers)
    in_maps = _prep_inputs(inputs, n_layers)
    res = run_bass_kernel_spmd(
        nc, in_maps, core_ids=list(range(NCORE)), trace=trace,
    )
    outs = [np.asarray(r["out"], dtype=np.float32) for r in res.results]
    full = np.concatenate(outs, axis=1)[:, :V]          # [2048, V], rows (s, b, i)
    logits = full.reshape(8, B, 128, V).transpose(1, 0, 2, 3).reshape(B, T, V)
    return logits, res


def kernel(**inputs):
    logits, _ = run(inputs)
    return logits
